# revision 9
# baseline (speedup 1.0000x reference)
"""Barlow Twins loss on 8 trn2 NeuronCores.

Math: with A = normalize(z_a), B = normalize(z_b) (per-column, ddof=1) and
c = A.T @ B / N, the loss is

    loss = lam * sum(c**2) + sum_d [ (c_dd - 1)**2 - lam * c_dd**2 ]

sum(c**2) = ||A.T B||_F^2 / N^2 = tr((A A.T)(B B.T)) / N^2 needs only the
[N, N] Gram matrices Ga = A A.T, Gb = B B.T (N=256), which are separable over
column shards: Ga = sum_cores A_i A_i.T.  The diagonal c_dd comes from the
raw per-column dot r_d = sum_n a b plus the host-side correction
c_dd = (r_d - N mu_a mu_b) * istd_a * istd_b / N.

Per core (1024-column slice, transposed [D_local, N] layout, d = 8p + i so
each partition reads one contiguous 8KB run per DMA):
  - stats: s via one 3D vector-reduce per tensor, sum-of-squares via scalar
    engine square+accumulate per 128x256 tile, batched [128,8] small ops
    spread over gpsimd/vector/scalar
  - normalize to bf16 tiles (half on vector, half on scalar engine)
  - Gram via 32 bf16 PE matmuls accumulated in 4 PSUM tiles
  - raw diag products on gpsimd + one 3D vector-reduce
Host sums the 8 partial Grams and finishes the tiny [256,256] reduction in
float64.
"""

from contextlib import ExitStack

import numpy as np

N = 256
D = 8192
NCORES = 8
D_LOCAL = D // NCORES  # 1024
P = 128
NT = D_LOCAL // P  # 8 tiles per tensor per core
LAMBDA = 0.005

_CACHE: dict = {}


def _build_program():
    import concourse.bacc as bacc
    import concourse.tile as tile
    from concourse import mybir

    f32 = mybir.dt.float32
    bf16 = mybir.dt.bfloat16
    Alu = mybir.AluOpType
    Act = mybir.ActivationFunctionType
    X = mybir.AxisListType.X

    nc = bacc.Bacc("TRN2", target_bir_lowering=False, debug=False)

    za_t = nc.dram_tensor("za_t", [D_LOCAL, N], f32, kind="ExternalInput").ap()
    zb_t = nc.dram_tensor("zb_t", [D_LOCAL, N], f32, kind="ExternalInput").ap()
    ga = nc.dram_tensor("ga", [2, P, N], f32, kind="ExternalOutput").ap()
    gb = nc.dram_tensor("gb", [2, P, N], f32, kind="ExternalOutput").ap()
    qd = nc.dram_tensor("qd", [P, NT], f32, kind="ExternalOutput").ap()
    # packed per-column stats: [mean_a | var_a | mean_b | var_b]
    st_out = nc.dram_tensor("st_out", [P, 4 * NT], f32, kind="ExternalOutput").ap()

    # DRAM side view: d = p*NT + i  ->  [p, i, n]
    srcs = {
        "a": za_t.rearrange("(p i) n -> p i n", i=NT),
        "b": zb_t.rearrange("(p i) n -> p i n", i=NT),
    }

    with tile.TileContext(nc) as tc, ExitStack() as ctx:
        raw_pool = ctx.enter_context(tc.tile_pool(name="raw", bufs=1))
        zn_pool = ctx.enter_context(tc.tile_pool(name="zn", bufs=NT))
        sq_pool = ctx.enter_context(tc.tile_pool(name="sq", bufs=2))
        st_pool = ctx.enter_context(tc.tile_pool(name="st", bufs=1))
        out_pool = ctx.enter_context(tc.tile_pool(name="out", bufs=1))
        ps_pool = ctx.enter_context(tc.tile_pool(name="ps", bufs=1, space="PSUM"))

        raw = {}
        for t in ("a", "b"):
            raw[t] = raw_pool.tile([P, NT, N], f32, tag=f"raw{t}", name=f"raw{t}")
            # 4 partition-sliced DMAs -> 8KB contiguous per partition each
            for c in range(4):
                pr = slice(c * 32, (c + 1) * 32)
                nc.sync.dma_start(raw[t][pr, :, :], srcs[t][pr, :, :])

        prod = raw_pool.tile([P, NT, N], f32, tag="prod", name="prod")
        stats = st_pool.tile([P, 4 * NT], f32, tag="stats", name="stats")
        q_sb = out_pool.tile([P, NT], f32, tag="q", name="q_sb")
        ga_sb = out_pool.tile([P, 2, N], f32, tag="ga_sb", name="ga_sb")
        gb_sb = out_pool.tile([P, 2, N], f32, tag="gb_sb", name="gb_sb")
        ga_ps = [ps_pool.tile([P, N], f32, tag=f"ga{m}", name=f"ga_ps{m}") for m in range(2)]
        gb_ps = [ps_pool.tile([P, N], f32, tag=f"gb{m}", name=f"gb_ps{m}") for m in range(2)]

        # ---- stats per tensor (batched [128, NT] small ops) ----
        mean = {}
        istd = {}
        nbias = {}
        for ti, t in enumerate(("a", "b")):
            s = st_pool.tile([P, NT], f32, tag=f"s{t}", name=f"s{t}")
            nc.vector.reduce_sum(s[:], raw[t][:], axis=X)

            ssq = st_pool.tile([P, NT], f32, tag=f"ssq{t}", name=f"ssq{t}")
            for i in range(NT):
                sq = sq_pool.tile([P, N], f32, tag="sq", name=f"sq{t}{i}")
                nc.scalar.activation(
                    sq[:], raw[t][:, i, :], Act.Square, accum_out=ssq[:, i : i + 1]
                )

            mn = stats[:, ti * 2 * NT : ti * 2 * NT + NT]
            nc.gpsimd.tensor_scalar_mul(mn, s[:], 1.0 / N)
            # var = (ssq - s*mean) / (N-1)
            t0 = st_pool.tile([P, NT], f32, tag=f"t0{t}", name=f"t0{t}")
            nc.gpsimd.tensor_tensor(t0[:], s[:], mn, op=Alu.mult)
            v1_ = st_pool.tile([P, NT], f32, tag=f"v1{t}", name=f"v1{t}")
            nc.gpsimd.tensor_tensor(v1_[:], ssq[:], t0[:], op=Alu.subtract)
            var = stats[:, ti * 2 * NT + NT : ti * 2 * NT + 2 * NT]
            nc.gpsimd.tensor_scalar_mul(var, v1_[:], 1.0 / (N - 1))
            iv = st_pool.tile([P, NT], f32, tag=f"iv{t}", name=f"iv{t}")
            nc.vector.reciprocal(iv[:], var)
            sd = st_pool.tile([P, NT], f32, tag=f"sd{t}", name=f"sd{t}")
            nc.scalar.sqrt(sd[:], iv[:])
            nbm = st_pool.tile([P, NT], f32, tag=f"nbm{t}", name=f"nbm{t}")
            nc.gpsimd.tensor_tensor(nbm[:], mn, sd[:], op=Alu.mult)
            nb = st_pool.tile([P, NT], f32, tag=f"nb{t}", name=f"nb{t}")
            nc.gpsimd.tensor_scalar_mul(nb[:], nbm[:], -1.0)
            mean[t], istd[t], nbias[t] = mn, sd, nb

        # ---- normalize to bf16 (split DVE / ACT) ----
        zn = {}
        for t in ("a", "b"):
            for i in range(NT):
                nt_ = zn_pool.tile([P, N], bf16, tag=f"zn{t}", name=f"zn{t}{i}")
                if i % 2 == 0:
                    nc.vector.tensor_scalar(
                        out=nt_[:], in0=raw[t][:, i, :],
                        scalar1=mean[t][:, i : i + 1], scalar2=istd[t][:, i : i + 1],
                        op0=Alu.subtract, op1=Alu.mult,
                    )
                else:
                    nc.scalar.activation(
                        nt_[:], raw[t][:, i, :], Act.Identity,
                        bias=nbias[t][:, i : i + 1], scale=istd[t][:, i : i + 1],
                    )
                zn[(t, i)] = nt_

        for i in range(NT):
            first, last = i == 0, i == NT - 1
            for m in range(2):
                nc.tensor.matmul(
                    ga_ps[m][:], lhsT=zn[("a", i)][:, m * P : (m + 1) * P],
                    rhs=zn[("a", i)][:], start=first, stop=last,
                )
                nc.tensor.matmul(
                    gb_ps[m][:], lhsT=zn[("b", i)][:, m * P : (m + 1) * P],
                    rhs=zn[("b", i)][:], start=first, stop=last,
                )

        # ---- raw diag products on gpsimd + one reduce ----
        for i in range(NT):
            nc.gpsimd.tensor_tensor(
                prod[:, i, :], raw["a"][:, i, :], raw["b"][:, i, :], op=Alu.mult
            )
        nc.vector.reduce_sum(q_sb[:], prod[:], axis=X)

        # ---- outputs ----
        nc.vector.tensor_copy(ga_sb[:, 0, :], ga_ps[0][:])
        nc.scalar.copy(ga_sb[:, 1, :], ga_ps[1][:])
        nc.vector.tensor_copy(gb_sb[:, 0, :], gb_ps[0][:])
        nc.scalar.copy(gb_sb[:, 1, :], gb_ps[1][:])
        for m in range(2):
            nc.sync.dma_start(ga[m], ga_sb[:, m, :])
            nc.sync.dma_start(gb[m], gb_sb[:, m, :])
        nc.sync.dma_start(qd[:], q_sb[:])
        nc.sync.dma_start(st_out[:], stats[:])

    nc.compile()
    return nc


def _get_program():
    if "nc" not in _CACHE:
        _CACHE["nc"] = _build_program()
    return _CACHE["nc"]


LAST_RESULT = None


def kernel(z_a: np.ndarray, z_b: np.ndarray) -> np.ndarray:
    global LAST_RESULT
    from concourse.bass_utils import run_bass_kernel_spmd

    z_a = np.asarray(z_a, dtype=np.float32)
    z_b = np.asarray(z_b, dtype=np.float32)
    assert z_a.shape == (N, D) and z_b.shape == (N, D)

    nc = _get_program()

    in_maps = []
    for c in range(NCORES):
        sl = slice(c * D_LOCAL, (c + 1) * D_LOCAL)
        in_maps.append(
            {
                "za_t": np.ascontiguousarray(z_a[:, sl].T),
                "zb_t": np.ascontiguousarray(z_b[:, sl].T),
            }
        )

    res = run_bass_kernel_spmd(nc, in_maps, core_ids=list(range(NCORES)))
    LAST_RESULT = res

    Ga = np.zeros((2 * P, N), dtype=np.float64)
    Gb = np.zeros((2 * P, N), dtype=np.float64)
    q = np.empty(D, dtype=np.float64)  # c_dd * N
    for c in range(NCORES):
        out = res.results[c]
        Ga += out["ga"].reshape(2 * P, N).astype(np.float64)
        Gb += out["gb"].reshape(2 * P, N).astype(np.float64)
        st = out["st_out"].astype(np.float64)
        mean_a, var_a = st[:, 0:NT], st[:, NT : 2 * NT]
        mean_b, var_b = st[:, 2 * NT : 3 * NT], st[:, 3 * NT : 4 * NT]
        r = out["qd"].astype(np.float64)  # [P, NT], raw sum_n a*b at (p, i)
        qc = (r - N * mean_a * mean_b) / np.sqrt(var_a * var_b)
        # d_local = p*NT + i -> row-major flatten
        q[c * D_LOCAL : (c + 1) * D_LOCAL] = qc.reshape(D_LOCAL)

    sum_c2 = float((Ga * Gb).sum()) / (N * N)  # sum over ALL (d, e) of c^2
    cdd = q / N
    loss = (
        LAMBDA * (sum_c2 - float((cdd * cdd).sum()))
        + float(((cdd - 1.0) ** 2).sum())
    )
    return np.float32(loss)


if __name__ == "__main__":
    rng = np.random.default_rng(0)
    za = rng.standard_normal((N, D), dtype=np.float32)
    zb = rng.standard_normal((N, D), dtype=np.float32)
    out = kernel(z_a=za, z_b=zb)
    print("kernel output:", out)


# revision 17
# speedup vs baseline: 1.1489x; 1.1489x over previous
"""Barlow Twins loss on 8 trn2 NeuronCores — hand-scheduled Bass kernel.

Math: with A = normalize(z_a), B = normalize(z_b) (per-column, ddof=1) and
c = A.T @ B / N:

    loss = lam * sum(c**2) + sum_d [ (c_dd - 1)**2 - lam * c_dd**2 ]
    sum(c**2) = tr((A A.T)(B B.T)) / N^2      (Gram matrices are [N, N])

Ga = A A.T is separable over column shards (Ga = sum_cores A_i A_i.T), so each
core computes partial [256, 256] Grams over its 1024-column slice via PE
matmuls on bf16-normalized tiles, plus raw per-column dots r_d = sum_n a*b
(host corrects: c_dd = (r_d - N mu_a mu_b) * istd_a * istd_b / N) and
per-column mean/var.  The host reduces the 8 partials in float64.

The device program is raw per-engine code (no Tile): inputs arrive as bf16
[1024, 256] transposed slices (d = 8p + i), two half-DMAs per tensor on the
two HWDGE rings (sync ring = z_a, scalar ring = z_b); per-half stats chains
(vector reduces, scalar-engine squares/sqrt, small [128,4] vector ops);
normalized bf16 tiles feed 32 PE matmuls accumulated in 4 PSUM banks; diag
products run on gpsimd.  PE is pre-warmed with dummy matmuls so the real ones
run at high p-state.
"""

import numpy as np

N = 256
D = 8192
NCORES = 8
D_LOCAL = D // NCORES  # 1024
P = 128
NT = D_LOCAL // P  # 8 tiles per tensor per core
NH = NT // 2  # tiles per half
LAMBDA = 0.005

_CACHE: dict = {}

# norm engine assignment: vector engine does tiles (0, 1), scalar the rest.
DVE_I = (0, 1)
ACT_I = (2, 3, 4, 5, 6, 7)
# PE consumes scalar-normalized tiles first (they are ready earlier)
PE_ORDER = list(ACT_I) + list(DVE_I)
N_DUMMY_MM = 6


def _build_program(ev_in=None):
    ev_in = ev_in or {}
    import concourse.bacc as bacc
    from concourse import mybir

    f32 = mybir.dt.float32
    bf16 = mybir.dt.bfloat16
    Alu = mybir.AluOpType
    Act = mybir.ActivationFunctionType
    X = mybir.AxisListType.X

    nc = bacc.Bacc("TRN2", target_bir_lowering=False, debug=False)

    za_t = nc.dram_tensor("za_t", [D_LOCAL, N], bf16, kind="ExternalInput").ap()
    zb_t = nc.dram_tensor("zb_t", [D_LOCAL, N], bf16, kind="ExternalInput").ap()
    ga = nc.dram_tensor("ga", [2, P, N], f32, kind="ExternalOutput").ap()
    gb = nc.dram_tensor("gb", [2, P, N], f32, kind="ExternalOutput").ap()
    qd = nc.dram_tensor("qd", [P, NT], f32, kind="ExternalOutput").ap()
    # packed stats: [mean_a | var_a | mean_b | var_b], NT cols each
    st_out = nc.dram_tensor("st_out", [P, 4 * NT], f32, kind="ExternalOutput").ap()

    src = {
        "a": za_t.rearrange("(p i) n -> p i n", i=NT),
        "b": zb_t.rearrange("(p i) n -> p i n", i=NT),
    }

    # ---- SBUF / PSUM ----
    raw = {t: nc.alloc_sbuf_tensor(f"raw_{t}", [P, NT, N], bf16).ap() for t in "ab"}
    sq = {t: nc.alloc_sbuf_tensor(f"sq_{t}", [P, NT, N], bf16).ap() for t in "ab"}
    zn = {t: nc.alloc_sbuf_tensor(f"zn_{t}", [P, NT, N], bf16).ap() for t in "ab"}
    prod = nc.alloc_sbuf_tensor("prod", [P, NT, N], bf16).ap()
    s_ = {t: nc.alloc_sbuf_tensor(f"s_{t}", [P, NT], f32).ap() for t in "ab"}
    ssq = {t: nc.alloc_sbuf_tensor(f"ssq_{t}", [P, NT], f32).ap() for t in "ab"}
    t1 = {t: nc.alloc_sbuf_tensor(f"t1_{t}", [P, NT], f32).ap() for t in "ab"}
    v0 = {t: nc.alloc_sbuf_tensor(f"v0_{t}", [P, NT], f32).ap() for t in "ab"}
    iv = {t: nc.alloc_sbuf_tensor(f"iv_{t}", [P, NT], f32).ap() for t in "ab"}
    sd = {t: nc.alloc_sbuf_tensor(f"sd_{t}", [P, NT], f32).ap() for t in "ab"}
    nbm = {t: nc.alloc_sbuf_tensor(f"nbm_{t}", [P, NT], f32).ap() for t in "ab"}
    nb = {t: nc.alloc_sbuf_tensor(f"nb_{t}", [P, NT], f32).ap() for t in "ab"}
    stats = nc.alloc_sbuf_tensor("stats", [P, 4 * NT], f32).ap()
    q_sb = nc.alloc_sbuf_tensor("q_sb", [P, NT], f32).ap()
    g_sb = {t: nc.alloc_sbuf_tensor(f"g_sb_{t}", [P, 2, N], f32).ap() for t in "ab"}
    gps = {
        t: [nc.alloc_psum_tensor(f"g_ps_{t}{m}", [P, N], f32).ap() for m in range(2)]
        for t in "ab"
    }
    dummy_ps = nc.alloc_psum_tensor("dummy_ps", [P, N], f32).ap()
    dummy_sb = nc.alloc_sbuf_tensor("dummy_sb", [P, N], bf16).ap()

    mn = {"a": stats[:, 0:NT], "b": stats[:, 2 * NT : 3 * NT]}
    var = {"a": stats[:, NT : 2 * NT], "b": stats[:, 3 * NT : 4 * NT]}

    # ---- semaphores ----
    # One rolling "chain" semaphore per compute engine: every instruction on
    # that engine waits for the previous one to complete and increments the
    # chain.  Cross-engine dependencies wait on the producer engine's chain
    # value at the producer's index (vector-clock style).  DMA completions
    # get dedicated sems (HW increments by 16).
    sem = {
        name: nc.alloc_semaphore(name)
        for name in (
            "da0", "da1", "db0", "db1",
            "vch", "ach", "pch",
            "mma", "mmb", "dout_s", "dout_a",
        )
    }
    dmas = {("a", 0): sem["da0"], ("a", 1): sem["da1"],
            ("b", 0): sem["db0"], ("b", 1): sem["db1"]}
    mms = {"a": sem["mma"], "b": sem["mmb"]}

    cnt = {"v": 0, "a": 0, "p": 0}
    chain = {"v": sem["vch"], "a": sem["ach"], "p": sem["pch"]}
    ev = {}  # event name -> (engine key, chain value when complete)

    def em(ek, ins, event=None):
        ins._wait_ge(chain[ek], cnt[ek])
        ins.then_inc(chain[ek], 1)
        cnt[ek] += 1
        if event:
            ev[event] = (ek, cnt[ek])
        return ins

    def wait_ev(eng, ek, event):
        val = ev_in.get(event, (ek, 0))[1]
        eng.wait_ge(chain[ek], val)

    def cs(h):  # stats-column slice of half h
        return slice(h * NH, (h + 1) * NH)

    def tsl(h):  # tile slice of half h
        return slice(h * NH, (h + 1) * NH)

    with nc.Block() as block:
        # NOTE: block bodies run in decorator order; events consumed by an
        # earlier-emitted engine must be produced by a later block only if
        # the ev[] entry already exists -> emit vector/scalar/gpsimd first,
        # then tensor, then sync (which only consumes).

        @block.vector
        def _(vector):
            kN = 1.0 / N
            kV = 1.0 / (N - 1)
            for t in "ab":
                for h in range(2):
                    c = cs(h)
                    nc.vector.wait_ge(dmas[(t, h)], 16)
                    em("v", nc.vector.reduce_sum(
                        s_[t][:, c], raw[t][:, tsl(h), :], axis=X))
                    wait_ev(nc.vector, "a", f"sq_{t}{h}")
                    em("v", nc.vector.reduce_sum(
                        ssq[t][:, c], sq[t][:, tsl(h), :], axis=X))
                    em("v", nc.vector.tensor_scalar_mul(mn[t][:, c], s_[t][:, c], kN))
                    em("v", nc.vector.tensor_tensor(
                        t1[t][:, c], s_[t][:, c], mn[t][:, c], op=Alu.mult))
                    em("v", nc.vector.tensor_tensor(
                        v0[t][:, c], ssq[t][:, c], t1[t][:, c], op=Alu.subtract))
                    em("v", nc.vector.tensor_scalar_mul(
                        var[t][:, c], v0[t][:, c], kV), event=f"var_{t}{h}")
                    em("v", nc.vector.reciprocal(
                        iv[t][:, c], var[t][:, c]), event=f"iv_{t}{h}")
            # vector-side norms (tiles 0, 1; half-0 stats)
            for t in "ab":
                wait_ev(nc.vector, "a", f"istd_{t}0")
                for i in DVE_I:
                    em("v", nc.vector.tensor_scalar(
                        out=zn[t][:, i, :], in0=raw[t][:, i, :],
                        scalar1=mn[t][:, i : i + 1], scalar2=sd[t][:, i : i + 1],
                        op0=Alu.subtract, op1=Alu.mult,
                    ), event=f"norm_{t}{i}")
            # diag reduces
            for h in range(2):
                wait_ev(nc.vector, "p", f"prod{h}")
                em("v", nc.vector.reduce_sum(
                    q_sb[:, cs(h)], prod[:, tsl(h), :], axis=X),
                    event=f"qred{h}")
            # psum copies (m=0)
            for t in "ab":
                nc.vector.wait_ge(mms[t], 1)
                em("v", nc.vector.tensor_copy(
                    g_sb[t][:, 0, :], gps[t][0][:]), event=f"cp0_{t}")

        @block.scalar
        def _(scalar):
            for h in range(2):
                nc.scalar.dma_start(
                    raw["b"][:, tsl(h), :], src["b"][:, tsl(h), :]
                ).then_inc(dmas[("b", h)], 16)
            # squares, in DMA-arrival order
            for h in range(2):
                for t in "ab":
                    nc.scalar.wait_ge(dmas[(t, h)], 16)
                    em("a", nc.scalar.activation(
                        sq[t][:, tsl(h), :], raw[t][:, tsl(h), :], Act.Square
                    ), event=f"sq_{t}{h}")
            # per-half sqrt + norms
            for t in "ab":
                for h in range(2):
                    wait_ev(nc.scalar, "v", f"iv_{t}{h}")
                    em("a", nc.scalar.sqrt(
                        sd[t][:, cs(h)], iv[t][:, cs(h)]), event=f"istd_{t}{h}")
                    wait_ev(nc.scalar, "p", f"nb_{t}{h}")
                    for i in ACT_I:
                        if i // NH != h:
                            continue
                        em("a", nc.scalar.activation(
                            zn[t][:, i, :], raw[t][:, i, :], Act.Identity,
                            bias=nb[t][:, i : i + 1], scale=sd[t][:, i : i + 1],
                        ), event=f"norm_{t}{i}")
            # psum copies (m=1) + gb out on this ring
            for t in "ab":
                nc.scalar.wait_ge(mms[t], 2)
                em("a", nc.scalar.copy(
                    g_sb[t][:, 1, :], gps[t][1][:]), event=f"cp1_{t}")
            wait_ev(nc.scalar, "v", "cp0_b")
            wait_ev(nc.scalar, "a", "cp1_b")
            nc.scalar.dma_start(
                gb.rearrange("m p n -> p m n"), g_sb["b"][:]
            ).then_inc(sem["dout_a"], 16)
            nc.scalar.wait_ge(sem["dout_a"], 16)

        @block.gpsimd
        def _(gpsimd):
            em("p", nc.gpsimd.memset(dummy_sb[:], 0.0), event="dumz")
            # nb = -(mean * istd) per half; diag products between chains
            for h in range(2):
                c = cs(h)
                wait_ev(nc.gpsimd, "a", f"istd_a{h}")
                em("p", nc.gpsimd.tensor_tensor(
                    nbm["a"][:, c], mn["a"][:, c], sd["a"][:, c], op=Alu.mult))
                em("p", nc.gpsimd.tensor_scalar_mul(
                    nb["a"][:, c], nbm["a"][:, c], -1.0), event=f"nb_a{h}")
            nc.gpsimd.wait_ge(sem["da0"], 16)
            nc.gpsimd.wait_ge(sem["db0"], 16)
            em("p", nc.gpsimd.tensor_tensor(
                prod[:, tsl(0), :], raw["a"][:, tsl(0), :], raw["b"][:, tsl(0), :],
                op=Alu.mult), event="prod0")
            for h in range(2):
                c = cs(h)
                wait_ev(nc.gpsimd, "a", f"istd_b{h}")
                em("p", nc.gpsimd.tensor_tensor(
                    nbm["b"][:, c], mn["b"][:, c], sd["b"][:, c], op=Alu.mult))
                em("p", nc.gpsimd.tensor_scalar_mul(
                    nb["b"][:, c], nbm["b"][:, c], -1.0), event=f"nb_b{h}")
            nc.gpsimd.wait_ge(sem["da1"], 16)
            nc.gpsimd.wait_ge(sem["db1"], 16)
            em("p", nc.gpsimd.tensor_tensor(
                prod[:, tsl(1), :], raw["a"][:, tsl(1), :], raw["b"][:, tsl(1), :],
                op=Alu.mult), event="prod1")

        @block.tensor
        def _(tensor):
            # p-state warmup: dummy matmuls on zeroed scratch
            wait_ev(nc.tensor, "p", "dumz")
            for _i in range(N_DUMMY_MM):
                nc.tensor.matmul(
                    dummy_ps[:], lhsT=dummy_sb[:, 0:P], rhs=dummy_sb[:],
                    start=True, stop=True, skip_group_check=True,
                )
            for t in "ab":
                for idx, i in enumerate(PE_ORDER):
                    wait_ev(nc.tensor, "v" if i in DVE_I else "a", f"norm_{t}{i}")
                    first, last = idx == 0, idx == NT - 1
                    for m in range(2):
                        ins = nc.tensor.matmul(
                            gps[t][m][:], lhsT=zn[t][:, i, m * P : (m + 1) * P],
                            rhs=zn[t][:, i, :], start=first, stop=last,
                        )
                        if last:
                            ins.then_inc(mms[t], 1)

        @block.sync
        def _(sync):
            for h in range(2):
                nc.sync.dma_start(
                    raw["a"][:, tsl(h), :], src["a"][:, tsl(h), :]
                ).then_inc(dmas[("a", h)], 16)
            # outputs, earliest-ready first
            wait_ev(nc.sync, "v", "var_b1")
            nc.sync.dma_start(st_out[:], stats[:]).then_inc(sem["dout_s"], 16)
            wait_ev(nc.sync, "v", "qred1")
            nc.sync.dma_start(qd[:], q_sb[:]).then_inc(sem["dout_s"], 16)
            wait_ev(nc.sync, "v", "cp0_a")
            wait_ev(nc.sync, "a", "cp1_a")
            nc.sync.dma_start(
                ga.rearrange("m p n -> p m n"), g_sb["a"][:]
            ).then_inc(sem["dout_s"], 16)
            nc.sync.wait_ge(sem["dout_s"], 48)

    nc.compile()
    return nc, ev


def _get_program():
    if "nc" not in _CACHE:
        _, ev = _build_program()       # pass 1: record event chain indices
        _CACHE["nc"], _ = _build_program(ev)  # pass 2: real wait values
    return _CACHE["nc"]


LAST_RESULT = None


def kernel(z_a: np.ndarray, z_b: np.ndarray) -> np.ndarray:
    global LAST_RESULT
    import ml_dtypes

    from concourse.bass_utils import run_bass_kernel_spmd

    z_a = np.asarray(z_a, dtype=np.float32)
    z_b = np.asarray(z_b, dtype=np.float32)
    assert z_a.shape == (N, D) and z_b.shape == (N, D)

    nc = _get_program()

    bf = ml_dtypes.bfloat16
    in_maps = []
    for c in range(NCORES):
        sl = slice(c * D_LOCAL, (c + 1) * D_LOCAL)
        in_maps.append(
            {
                "za_t": np.ascontiguousarray(z_a[:, sl].T.astype(bf)),
                "zb_t": np.ascontiguousarray(z_b[:, sl].T.astype(bf)),
            }
        )

    res = run_bass_kernel_spmd(nc, in_maps, core_ids=list(range(NCORES)))
    LAST_RESULT = res

    Ga = np.zeros((2 * P, N), dtype=np.float64)
    Gb = np.zeros((2 * P, N), dtype=np.float64)
    q = np.empty(D, dtype=np.float64)  # c_dd * N
    for c in range(NCORES):
        out = res.results[c]
        Ga += out["ga"].reshape(2 * P, N).astype(np.float64)
        Gb += out["gb"].reshape(2 * P, N).astype(np.float64)
        st = out["st_out"].astype(np.float64)
        mean_a, var_a = st[:, 0:NT], st[:, NT : 2 * NT]
        mean_b, var_b = st[:, 2 * NT : 3 * NT], st[:, 3 * NT : 4 * NT]
        r = out["qd"].astype(np.float64)  # [P, NT] raw sum_n a*b at (p, i)
        qc = (r - N * mean_a * mean_b) / np.sqrt(var_a * var_b)
        q[c * D_LOCAL : (c + 1) * D_LOCAL] = qc.reshape(D_LOCAL)

    sum_c2 = float((Ga * Gb).sum()) / (N * N)  # sum over ALL (d, e) of c^2
    cdd = q / N
    loss = (
        LAMBDA * (sum_c2 - float((cdd * cdd).sum()))
        + float(((cdd - 1.0) ** 2).sum())
    )
    return np.float32(loss)


if __name__ == "__main__":
    rng = np.random.default_rng(0)
    za = rng.standard_normal((N, D), dtype=np.float32)
    zb = rng.standard_normal((N, D), dtype=np.float32)
    out = kernel(z_a=za, z_b=zb)
    print("kernel output:", out)


# revision 19
# speedup vs baseline: 1.2161x; 1.0585x over previous
"""Barlow Twins loss on 8 trn2 NeuronCores — hand-scheduled Bass kernel.

Math: with A = normalize(z_a), B = normalize(z_b) (per-column, ddof=1) and
c = A.T @ B / N:

    loss = lam * sum(c**2) + sum_d [ (c_dd - 1)**2 - lam * c_dd**2 ]
    sum(c**2) = tr((A A.T)(B B.T)) / N^2      (Gram matrices are [N, N])

Ga = A A.T is separable over column shards (Ga = sum_cores A_i A_i.T), so each
core computes partial [256, 256] Grams over its 1024-column slice via PE
matmuls on bf16-normalized tiles, plus raw per-column dots r_d = sum_n a*b
(host corrects: c_dd = (r_d - N mu_a mu_b) * istd_a * istd_b / N) and
per-column mean/var.  The host reduces the 8 partials in float64.

The device program is raw per-engine code (no Tile): inputs arrive as bf16
[1024, 256] transposed slices (d = 8p + i), two half-DMAs per tensor on the
two HWDGE rings (sync ring = z_a, scalar ring = z_b); per-half stats chains
(vector reduces, scalar-engine squares/sqrt, small [128,4] vector ops);
normalized bf16 tiles feed 32 PE matmuls accumulated in 4 PSUM banks; diag
products run on gpsimd.  PE is pre-warmed with dummy matmuls so the real ones
run at high p-state.
"""

import numpy as np

N = 256
D = 8192
NCORES = 8
D_LOCAL = D // NCORES  # 1024
P = 128
NT = D_LOCAL // P  # 8 tiles per tensor per core
NH = NT // 2  # tiles per half
LAMBDA = 0.005

_CACHE: dict = {}

# norm engine assignment: vector engine does tiles (0, 1), scalar the rest.
DVE_I = (0, 1)
ACT_I = (2, 3, 4, 5, 6, 7)
# PE consumes scalar-normalized tiles first (they are ready earlier)
PE_ORDER = list(ACT_I) + list(DVE_I)
N_DUMMY_MM = 6


def _build_program(ev_in=None):
    ev_in = ev_in or {}
    import concourse.bacc as bacc
    from concourse import mybir

    f32 = mybir.dt.float32
    bf16 = mybir.dt.bfloat16
    Alu = mybir.AluOpType
    Act = mybir.ActivationFunctionType
    X = mybir.AxisListType.X

    nc = bacc.Bacc("TRN2", target_bir_lowering=False, debug=False)

    za_t = nc.dram_tensor("za_t", [D_LOCAL, N], bf16, kind="ExternalInput").ap()
    zb_t = nc.dram_tensor("zb_t", [D_LOCAL, N], bf16, kind="ExternalInput").ap()
    ga = nc.dram_tensor("ga", [2, P, N], f32, kind="ExternalOutput").ap()
    gb = nc.dram_tensor("gb", [2, P, N], f32, kind="ExternalOutput").ap()
    qd = nc.dram_tensor("qd", [P, NT], f32, kind="ExternalOutput").ap()
    # per-tensor stats: [..., 0] = mean, [..., 1] = biased var
    st_a = nc.dram_tensor("st_a", [P, NT, 2], f32, kind="ExternalOutput").ap()
    st_b = nc.dram_tensor("st_b", [P, NT, 2], f32, kind="ExternalOutput").ap()

    src = {
        "a": za_t.rearrange("(p i) n -> p i n", i=NT),
        "b": zb_t.rearrange("(p i) n -> p i n", i=NT),
    }

    # ---- SBUF / PSUM ----
    raw = {t: nc.alloc_sbuf_tensor(f"raw_{t}", [P, NT, N], bf16).ap() for t in "ab"}
    zn = {t: nc.alloc_sbuf_tensor(f"zn_{t}", [P, NT, N], bf16).ap() for t in "ab"}
    prod = nc.alloc_sbuf_tensor("prod", [P, NT, N], bf16).ap()
    bnst = {t: nc.alloc_sbuf_tensor(f"bnst_{t}", [P, NT, 6], f32).ap() for t in "ab"}
    smv = {t: nc.alloc_sbuf_tensor(f"smv_{t}", [P, NT, 2], f32).ap() for t in "ab"}
    iv = {t: nc.alloc_sbuf_tensor(f"iv_{t}", [P, NT], f32).ap() for t in "ab"}
    sd = {t: nc.alloc_sbuf_tensor(f"sd_{t}", [P, NT], f32).ap() for t in "ab"}
    nbm = {t: nc.alloc_sbuf_tensor(f"nbm_{t}", [P, NT], f32).ap() for t in "ab"}
    nb = {t: nc.alloc_sbuf_tensor(f"nb_{t}", [P, NT], f32).ap() for t in "ab"}
    q_sb = nc.alloc_sbuf_tensor("q_sb", [P, NT], f32).ap()
    g_sb = {t: nc.alloc_sbuf_tensor(f"g_sb_{t}", [P, 2, N], f32).ap() for t in "ab"}
    scr1 = nc.alloc_sbuf_tensor("scr1", [P, 1], f32).ap()
    gps = {
        t: [nc.alloc_psum_tensor(f"g_ps_{t}{m}", [P, N], f32).ap() for m in range(2)]
        for t in "ab"
    }
    dummy_ps = nc.alloc_psum_tensor("dummy_ps", [P, N], f32).ap()
    dummy_sb = nc.alloc_sbuf_tensor("dummy_sb", [P, N], bf16).ap()

    def mn(t, i):  # [P, 1] mean column for tile i
        return smv[t][:, i, 0:1]

    # ---- semaphores ----
    # One rolling "chain" semaphore per compute engine; cross-engine deps
    # wait on the producer engine's chain value at the producer's index.
    sem = {
        name: nc.alloc_semaphore(name)
        for name in (
            "da0", "da1", "db0", "db1",
            "vch", "ach", "pch",
            "mma", "mmb", "dout_s", "dout_a",
        )
    }
    dmas = {("a", 0): sem["da0"], ("a", 1): sem["da1"],
            ("b", 0): sem["db0"], ("b", 1): sem["db1"]}
    mms = {"a": sem["mma"], "b": sem["mmb"]}

    cnt = {"v": 0, "a": 0, "p": 0}
    chain = {"v": sem["vch"], "a": sem["ach"], "p": sem["pch"]}
    ev = {}

    def em(ek, ins, event=None):
        ins._wait_ge(chain[ek], cnt[ek])
        ins.then_inc(chain[ek], 1)
        cnt[ek] += 1
        if event:
            ev[event] = (ek, cnt[ek])
        return ins

    def wait_ev(eng, ek, event):
        val = ev_in.get(event, (ek, 0))[1]
        eng.wait_ge(chain[ek], val)

    def tsl(h):  # tile slice of half h
        return slice(h * NH, (h + 1) * NH)

    # PE consumption order (tile ready-time order)
    PE_SCHED = [("a", 2), ("a", 3), ("b", 2), ("b", 3), ("a", 4), ("a", 5),
                ("a", 0), ("a", 1), ("a", 6), ("a", 7), ("b", 4), ("b", 5),
                ("b", 0), ("b", 1), ("b", 6), ("b", 7)]
    first_tile = {"a": ("a", 2), "b": ("b", 2)}
    last_tile = {"a": ("a", 7), "b": ("b", 7)}

    with nc.Block() as block:

        @block.vector
        def _(vector):
            # per-half bn stats: 2x bn_stats (512 free each), 4x bn_aggr,
            # then reciprocal of the biased var -> iv
            for h in range(2):
                for t in "ab":
                    nc.vector.wait_ge(dmas[(t, h)], 16)
                    for i in range(h * NH, (h + 1) * NH):
                        em("v", nc.vector.bn_stats(
                            bnst[t][:, i, :], raw[t][:, i, :]))
                    for i in range(h * NH, (h + 1) * NH):
                        em("v", nc.vector.bn_aggr(
                            smv[t][:, i, :], bnst[t][:, i, :]),
                            event=f"bn_{t}{h}" if i == (h + 1) * NH - 1 else None)
                    em("v", nc.vector.reciprocal(
                        iv[t][:, tsl(h)], smv[t][:, tsl(h), 1]),
                        event=f"iv_{t}{h}")
            # vector-side norms (tiles 0, 1; half-0 stats)
            for t in "ab":
                wait_ev(nc.vector, "a", f"istd_{t}0")
                for i in DVE_I:
                    em("v", nc.vector.tensor_scalar(
                        out=zn[t][:, i, :], in0=raw[t][:, i, :],
                        scalar1=mn(t, i), scalar2=sd[t][:, i : i + 1],
                        op0=Alu.subtract, op1=Alu.mult,
                    ), event=f"norm_{t}{i}")
            # diag reduces + psum copies, by availability
            wait_ev(nc.vector, "p", "prod0")
            em("v", nc.vector.reduce_sum(
                q_sb[:, 0:NH], prod[:, tsl(0), :], axis=X))
            nc.vector.wait_ge(mms["a"], 1)
            em("v", nc.vector.tensor_copy(
                g_sb["a"][:, 0, :], gps["a"][0][:]), event="cp0_a")
            wait_ev(nc.vector, "p", "prod1")
            em("v", nc.vector.reduce_sum(
                q_sb[:, NH:NT], prod[:, tsl(1), :], axis=X), event="qred1")
            nc.vector.wait_ge(mms["b"], 1)
            em("v", nc.vector.tensor_copy(
                g_sb["b"][:, 0, :], gps["b"][0][:]), event="cp0_b")

        @block.scalar
        def _(scalar):
            for h in range(2):
                nc.scalar.dma_start(
                    raw["b"][:, tsl(h), :], src["b"][:, tsl(h), :]
                ).then_inc(dmas[("b", h)], 16)
            # preload ACT tables (Sqrt + Identity) while DMAs fly
            em("a", nc.scalar.sqrt(scr1[:], nc.const_aps.scalar_like(1.0, scr1)))
            em("a", nc.scalar.activation(scr1[:], scr1[:], Act.Identity))
            # per-half sqrt(iv * (N-1)/N) -> unbiased istd, then norms
            kB = (N - 1.0) / N
            for h in range(2):
                for t in "ab":
                    wait_ev(nc.scalar, "v", f"iv_{t}{h}")
                    em("a", nc.scalar.activation(
                        sd[t][:, tsl(h)], iv[t][:, tsl(h)], Act.Sqrt, scale=kB),
                        event=f"istd_{t}{h}")
                    wait_ev(nc.scalar, "p", f"nb_{t}{h}")
                    for i in ACT_I:
                        if i // NH != h:
                            continue
                        em("a", nc.scalar.activation(
                            zn[t][:, i, :], raw[t][:, i, :], Act.Identity,
                            bias=nb[t][:, i : i + 1], scale=sd[t][:, i : i + 1],
                        ), event=f"norm_{t}{i}")
            # psum copies (m=1) + gb out on this ring
            for t in "ab":
                nc.scalar.wait_ge(mms[t], 2)
                em("a", nc.scalar.copy(
                    g_sb[t][:, 1, :], gps[t][1][:]), event=f"cp1_{t}")
            wait_ev(nc.scalar, "v", "cp0_b")
            wait_ev(nc.scalar, "a", "cp1_b")
            nc.scalar.dma_start(
                gb.rearrange("m p n -> p m n"), g_sb["b"][:]
            ).then_inc(sem["dout_a"], 16)
            nc.scalar.wait_ge(sem["dout_a"], 16)

        @block.gpsimd
        def _(gpsimd):
            em("p", nc.gpsimd.memset(dummy_sb[:], 0.0), event="dumz")
            # nb = -(mean * istd), per (tensor, half); diag products last
            for h in range(2):
                for t in "ab":
                    c = slice(h * NH, (h + 1) * NH)
                    wait_ev(nc.gpsimd, "a", f"istd_{t}{h}")
                    em("p", nc.gpsimd.tensor_tensor(
                        nbm[t][:, c], smv[t][:, c, 0], sd[t][:, c], op=Alu.mult))
                    em("p", nc.gpsimd.tensor_scalar_mul(
                        nb[t][:, c], nbm[t][:, c], -1.0), event=f"nb_{t}{h}")
            for h in range(2):
                nc.gpsimd.wait_ge(dmas[("a", h)], 16)
                nc.gpsimd.wait_ge(dmas[("b", h)], 16)
                em("p", nc.gpsimd.tensor_tensor(
                    prod[:, tsl(h), :], raw["a"][:, tsl(h), :],
                    raw["b"][:, tsl(h), :], op=Alu.mult), event=f"prod{h}")

        @block.tensor
        def _(tensor):
            # p-state warmup: dummy matmuls on zeroed scratch
            wait_ev(nc.tensor, "p", "dumz")
            for _i in range(N_DUMMY_MM):
                nc.tensor.matmul(
                    dummy_ps[:], lhsT=dummy_sb[:, 0:P], rhs=dummy_sb[:],
                    start=True, stop=True, skip_group_check=True,
                )
            for t, i in PE_SCHED:
                wait_ev(nc.tensor, "v" if i in DVE_I else "a", f"norm_{t}{i}")
                first = (t, i) == first_tile[t]
                last = (t, i) == last_tile[t]
                for m in range(2):
                    ins = nc.tensor.matmul(
                        gps[t][m][:], lhsT=zn[t][:, i, m * P : (m + 1) * P],
                        rhs=zn[t][:, i, :], start=first, stop=last,
                    )
                    if last:
                        ins.then_inc(mms[t], 1)

        @block.sync
        def _(sync):
            for h in range(2):
                nc.sync.dma_start(
                    raw["a"][:, tsl(h), :], src["a"][:, tsl(h), :]
                ).then_inc(dmas[("a", h)], 16)
            # outputs, earliest-ready first
            wait_ev(nc.sync, "v", "bn_a1")
            nc.sync.dma_start(st_a[:], smv["a"][:]).then_inc(sem["dout_s"], 16)
            wait_ev(nc.sync, "v", "bn_b1")
            nc.sync.dma_start(st_b[:], smv["b"][:]).then_inc(sem["dout_s"], 16)
            wait_ev(nc.sync, "v", "qred1")
            nc.sync.dma_start(qd[:], q_sb[:]).then_inc(sem["dout_s"], 16)
            wait_ev(nc.sync, "v", "cp0_a")
            wait_ev(nc.sync, "a", "cp1_a")
            nc.sync.dma_start(
                ga.rearrange("m p n -> p m n"), g_sb["a"][:]
            ).then_inc(sem["dout_s"], 16)
            nc.sync.wait_ge(sem["dout_s"], 64)

    nc.compile()
    return nc, ev


def _get_program():
    if "nc" not in _CACHE:
        _, ev = _build_program()       # pass 1: record event chain indices
        _CACHE["nc"], _ = _build_program(ev)  # pass 2: real wait values
    return _CACHE["nc"]


LAST_RESULT = None


def kernel(z_a: np.ndarray, z_b: np.ndarray) -> np.ndarray:
    global LAST_RESULT
    import ml_dtypes

    from concourse.bass_utils import run_bass_kernel_spmd

    z_a = np.asarray(z_a, dtype=np.float32)
    z_b = np.asarray(z_b, dtype=np.float32)
    assert z_a.shape == (N, D) and z_b.shape == (N, D)

    nc = _get_program()

    bf = ml_dtypes.bfloat16
    in_maps = []
    for c in range(NCORES):
        sl = slice(c * D_LOCAL, (c + 1) * D_LOCAL)
        in_maps.append(
            {
                "za_t": np.ascontiguousarray(z_a[:, sl].T.astype(bf)),
                "zb_t": np.ascontiguousarray(z_b[:, sl].T.astype(bf)),
            }
        )

    res = run_bass_kernel_spmd(nc, in_maps, core_ids=list(range(NCORES)))
    LAST_RESULT = res

    Ga = np.zeros((2 * P, N), dtype=np.float64)
    Gb = np.zeros((2 * P, N), dtype=np.float64)
    q = np.empty(D, dtype=np.float64)  # c_dd * N
    for c in range(NCORES):
        out = res.results[c]
        Ga += out["ga"].reshape(2 * P, N).astype(np.float64)
        Gb += out["gb"].reshape(2 * P, N).astype(np.float64)
        sta = out["st_a"].astype(np.float64)
        stb = out["st_b"].astype(np.float64)
        mean_a, var_a = sta[:, :, 0], sta[:, :, 1] * (N / (N - 1.0))
        mean_b, var_b = stb[:, :, 0], stb[:, :, 1] * (N / (N - 1.0))
        r = out["qd"].astype(np.float64)  # [P, NT] raw sum_n a*b at (p, i)
        qc = (r - N * mean_a * mean_b) / np.sqrt(var_a * var_b)
        q[c * D_LOCAL : (c + 1) * D_LOCAL] = qc.reshape(D_LOCAL)

    sum_c2 = float((Ga * Gb).sum()) / (N * N)  # sum over ALL (d, e) of c^2
    cdd = q / N
    loss = (
        LAMBDA * (sum_c2 - float((cdd * cdd).sum()))
        + float(((cdd - 1.0) ** 2).sum())
    )
    return np.float32(loss)


if __name__ == "__main__":
    rng = np.random.default_rng(0)
    za = rng.standard_normal((N, D), dtype=np.float32)
    zb = rng.standard_normal((N, D), dtype=np.float32)
    out = kernel(z_a=za, z_b=zb)
    print("kernel output:", out)


# revision 20
# speedup vs baseline: 1.2265x; 1.0085x over previous
"""Barlow Twins loss on 8 trn2 NeuronCores — hand-scheduled Bass kernel.

Math: with A = normalize(z_a), B = normalize(z_b) (per-column, ddof=1) and
c = A.T @ B / N:

    loss = lam * sum(c**2) + sum_d [ (c_dd - 1)**2 - lam * c_dd**2 ]
    sum(c**2) = tr((A A.T)(B B.T)) / N^2      (Gram matrices are [N, N])

Ga = A A.T is separable over column shards (Ga = sum_cores A_i A_i.T), so each
core computes partial [256, 256] Grams over its 1024-column slice via PE
matmuls on bf16-normalized tiles, plus raw per-column dots r_d = sum_n a*b
(host corrects: c_dd = (r_d - N mu_a mu_b) * istd_a * istd_b / N) and
per-column mean/var.  The host reduces the 8 partials in float64.

The device program is raw per-engine code (no Tile): inputs arrive as bf16
[1024, 256] transposed slices (d = 8p + i), two half-DMAs per tensor on the
two HWDGE rings (sync ring = z_a, scalar ring = z_b); per-half stats chains
(vector reduces, scalar-engine squares/sqrt, small [128,4] vector ops);
normalized bf16 tiles feed 32 PE matmuls accumulated in 4 PSUM banks; diag
products run on gpsimd.  PE is pre-warmed with dummy matmuls so the real ones
run at high p-state.
"""

import numpy as np

N = 256
D = 8192
NCORES = 8
D_LOCAL = D // NCORES  # 1024
P = 128
NT = D_LOCAL // P  # 8 tiles per tensor per core
NH = NT // 2  # tiles per half
LAMBDA = 0.005

_CACHE: dict = {}

# norm engine assignment: vector engine does tiles (0, 1), scalar the rest.
DVE_I = (0, 1)
ACT_I = (2, 3, 4, 5, 6, 7)
# PE consumes scalar-normalized tiles first (they are ready earlier)
PE_ORDER = list(ACT_I) + list(DVE_I)
N_DUMMY_MM = 12


def _build_program(ev_in=None):
    ev_in = ev_in or {}
    import concourse.bacc as bacc
    from concourse import mybir

    f32 = mybir.dt.float32
    bf16 = mybir.dt.bfloat16
    Alu = mybir.AluOpType
    Act = mybir.ActivationFunctionType
    X = mybir.AxisListType.X

    nc = bacc.Bacc("TRN2", target_bir_lowering=False, debug=False)

    za_t = nc.dram_tensor("za_t", [D_LOCAL, N], bf16, kind="ExternalInput").ap()
    zb_t = nc.dram_tensor("zb_t", [D_LOCAL, N], bf16, kind="ExternalInput").ap()
    ga = nc.dram_tensor("ga", [2, P, N], f32, kind="ExternalOutput").ap()
    gb = nc.dram_tensor("gb", [2, P, N], f32, kind="ExternalOutput").ap()
    qd = nc.dram_tensor("qd", [P, NT], f32, kind="ExternalOutput").ap()
    # per-tensor stats: [..., 0] = mean, [..., 1] = biased var
    st_a = nc.dram_tensor("st_a", [P, NT, 2], f32, kind="ExternalOutput").ap()
    st_b = nc.dram_tensor("st_b", [P, NT, 2], f32, kind="ExternalOutput").ap()

    src = {
        "a": za_t.rearrange("(p i) n -> p i n", i=NT),
        "b": zb_t.rearrange("(p i) n -> p i n", i=NT),
    }

    # ---- SBUF / PSUM ----
    raw = {t: nc.alloc_sbuf_tensor(f"raw_{t}", [P, NT, N], bf16).ap() for t in "ab"}
    zn = {t: nc.alloc_sbuf_tensor(f"zn_{t}", [P, NT, N], bf16).ap() for t in "ab"}
    prod = nc.alloc_sbuf_tensor("prod", [P, NT, N], bf16).ap()
    bnst = {t: nc.alloc_sbuf_tensor(f"bnst_{t}", [P, NT, 6], f32).ap() for t in "ab"}
    smv = {t: nc.alloc_sbuf_tensor(f"smv_{t}", [P, NT, 2], f32).ap() for t in "ab"}
    iv = {t: nc.alloc_sbuf_tensor(f"iv_{t}", [P, NT], f32).ap() for t in "ab"}
    sd = {t: nc.alloc_sbuf_tensor(f"sd_{t}", [P, NT], f32).ap() for t in "ab"}
    nbm = {t: nc.alloc_sbuf_tensor(f"nbm_{t}", [P, NT], f32).ap() for t in "ab"}
    nb = {t: nc.alloc_sbuf_tensor(f"nb_{t}", [P, NT], f32).ap() for t in "ab"}
    q_sb = nc.alloc_sbuf_tensor("q_sb", [P, NT], f32).ap()
    g_sb = {t: nc.alloc_sbuf_tensor(f"g_sb_{t}", [P, 2, N], f32).ap() for t in "ab"}
    scr1 = nc.alloc_sbuf_tensor("scr1", [P, 1], f32).ap()
    gps = {
        t: [nc.alloc_psum_tensor(f"g_ps_{t}{m}", [P, N], f32).ap() for m in range(2)]
        for t in "ab"
    }
    dummy_ps = nc.alloc_psum_tensor("dummy_ps", [P, N], f32).ap()
    dummy_sb = nc.alloc_sbuf_tensor("dummy_sb", [P, N], bf16).ap()

    def mn(t, i):  # [P, 1] mean column for tile i
        return smv[t][:, i, 0:1]

    # ---- semaphores ----
    # One rolling "chain" semaphore per compute engine; cross-engine deps
    # wait on the producer engine's chain value at the producer's index.
    sem = {
        name: nc.alloc_semaphore(name)
        for name in (
            "da0", "da1", "db0", "db1",
            "vch", "ach", "pch",
            "mma", "mmb", "dout_s", "dout_a",
        )
    }
    dmas = {("a", 0): sem["da0"], ("a", 1): sem["da1"],
            ("b", 0): sem["db0"], ("b", 1): sem["db1"]}
    mms = {"a": sem["mma"], "b": sem["mmb"]}

    cnt = {"v": 0, "a": 0, "p": 0}
    chain = {"v": sem["vch"], "a": sem["ach"], "p": sem["pch"]}
    ev = {}

    def em(ek, ins, event=None):
        ins._wait_ge(chain[ek], cnt[ek])
        ins.then_inc(chain[ek], 1)
        cnt[ek] += 1
        if event:
            ev[event] = (ek, cnt[ek])
        return ins

    def wait_ev(eng, ek, event):
        val = ev_in.get(event, (ek, 0))[1]
        eng.wait_ge(chain[ek], val)

    def tsl(h):  # tile slice of half h
        return slice(h * NH, (h + 1) * NH)

    # PE consumption order (tile ready-time order)
    PE_SCHED = [("a", 2), ("a", 3), ("b", 2), ("b", 3), ("a", 4), ("a", 5),
                ("a", 0), ("a", 1), ("a", 6), ("a", 7), ("b", 4), ("b", 5),
                ("b", 0), ("b", 1), ("b", 6), ("b", 7)]
    first_tile = {"a": ("a", 2), "b": ("b", 2)}
    last_tile = {"a": ("a", 7), "b": ("b", 7)}

    with nc.Block() as block:

        @block.vector
        def _(vector):
            # per-half bn stats: 2x bn_stats (512 free each), 4x bn_aggr,
            # then reciprocal of the biased var -> iv
            for h in range(2):
                for t in "ab":
                    nc.vector.wait_ge(dmas[(t, 0)], 16)
                    for i in range(h * NH, (h + 1) * NH):
                        em("v", nc.vector.bn_stats(
                            bnst[t][:, i, :], raw[t][:, i, :]))
                    for i in range(h * NH, (h + 1) * NH):
                        em("v", nc.vector.bn_aggr(
                            smv[t][:, i, :], bnst[t][:, i, :]),
                            event=f"bn_{t}{h}" if i == (h + 1) * NH - 1 else None)
                    em("v", nc.vector.reciprocal(
                        iv[t][:, tsl(h)], smv[t][:, tsl(h), 1]),
                        event=f"iv_{t}{h}")
            # vector-side norms (tiles 0, 1; half-0 stats)
            for t in "ab":
                wait_ev(nc.vector, "a", f"istd_{t}0")
                for i in DVE_I:
                    em("v", nc.vector.tensor_scalar(
                        out=zn[t][:, i, :], in0=raw[t][:, i, :],
                        scalar1=mn(t, i), scalar2=sd[t][:, i : i + 1],
                        op0=Alu.subtract, op1=Alu.mult,
                    ), event=f"norm_{t}{i}")
            # diag reduces + psum copies, by availability
            wait_ev(nc.vector, "p", "prod0")
            em("v", nc.vector.reduce_sum(
                q_sb[:, 0:NH], prod[:, tsl(0), :], axis=X))
            nc.vector.wait_ge(mms["a"], 1)
            em("v", nc.vector.tensor_copy(
                g_sb["a"][:, 0, :], gps["a"][0][:]), event="cp0_a")
            wait_ev(nc.vector, "p", "prod1")
            em("v", nc.vector.reduce_sum(
                q_sb[:, NH:NT], prod[:, tsl(1), :], axis=X), event="qred1")
            nc.vector.wait_ge(mms["b"], 1)
            em("v", nc.vector.tensor_copy(
                g_sb["b"][:, 0, :], gps["b"][0][:]), event="cp0_b")

        @block.scalar
        def _(scalar):
            nc.scalar.dma_start(raw["b"][:], src["b"][:]).then_inc(sem["db0"], 16)
            # preload ACT tables (Sqrt + Identity) while DMAs fly
            em("a", nc.scalar.sqrt(scr1[:], nc.const_aps.scalar_like(1.0, scr1)))
            em("a", nc.scalar.activation(scr1[:], scr1[:], Act.Identity))
            # per-half sqrt(iv * (N-1)/N) -> unbiased istd, then norms
            kB = (N - 1.0) / N
            for h in range(2):
                for t in "ab":
                    wait_ev(nc.scalar, "v", f"iv_{t}{h}")
                    em("a", nc.scalar.activation(
                        sd[t][:, tsl(h)], iv[t][:, tsl(h)], Act.Sqrt, scale=kB),
                        event=f"istd_{t}{h}")
                    wait_ev(nc.scalar, "p", f"nb_{t}{h}")
                    for i in ACT_I:
                        if i // NH != h:
                            continue
                        em("a", nc.scalar.activation(
                            zn[t][:, i, :], raw[t][:, i, :], Act.Identity,
                            bias=nb[t][:, i : i + 1], scale=sd[t][:, i : i + 1],
                        ), event=f"norm_{t}{i}")
            # psum copies (m=1) + gb out on this ring
            for t in "ab":
                nc.scalar.wait_ge(mms[t], 2)
                em("a", nc.scalar.copy(
                    g_sb[t][:, 1, :], gps[t][1][:]), event=f"cp1_{t}")
            wait_ev(nc.scalar, "v", "cp0_b")
            wait_ev(nc.scalar, "a", "cp1_b")
            nc.scalar.dma_start(
                gb.rearrange("m p n -> p m n"), g_sb["b"][:]
            ).then_inc(sem["dout_a"], 16)
            nc.scalar.wait_ge(sem["dout_a"], 16)

        @block.gpsimd
        def _(gpsimd):
            em("p", nc.gpsimd.memset(dummy_sb[:], 0.0), event="dumz")
            # diag products first (inputs arrive early, Pool otherwise idle)
            nc.gpsimd.wait_ge(sem["da0"], 16)
            nc.gpsimd.wait_ge(sem["db0"], 16)
            for h in range(2):
                em("p", nc.gpsimd.tensor_tensor(
                    prod[:, tsl(h), :], raw["a"][:, tsl(h), :],
                    raw["b"][:, tsl(h), :], op=Alu.mult), event=f"prod{h}")
            # nb = -(mean * istd), per (tensor, half)
            for h in range(2):
                for t in "ab":
                    c = slice(h * NH, (h + 1) * NH)
                    wait_ev(nc.gpsimd, "a", f"istd_{t}{h}")
                    em("p", nc.gpsimd.tensor_tensor(
                        nbm[t][:, c], smv[t][:, c, 0], sd[t][:, c], op=Alu.mult))
                    em("p", nc.gpsimd.tensor_scalar_mul(
                        nb[t][:, c], nbm[t][:, c], -1.0), event=f"nb_{t}{h}")
        @block.tensor
        def _(tensor):
            # p-state warmup: dummy matmuls on zeroed scratch
            wait_ev(nc.tensor, "p", "dumz")
            for _i in range(N_DUMMY_MM):
                nc.tensor.matmul(
                    dummy_ps[:], lhsT=dummy_sb[:, 0:P], rhs=dummy_sb[:],
                    start=True, stop=True, skip_group_check=True,
                )
            for t, i in PE_SCHED:
                wait_ev(nc.tensor, "v" if i in DVE_I else "a", f"norm_{t}{i}")
                first = (t, i) == first_tile[t]
                last = (t, i) == last_tile[t]
                for m in range(2):
                    ins = nc.tensor.matmul(
                        gps[t][m][:], lhsT=zn[t][:, i, m * P : (m + 1) * P],
                        rhs=zn[t][:, i, :], start=first, stop=last,
                    )
                    if last:
                        ins.then_inc(mms[t], 1)

        @block.sync
        def _(sync):
            nc.sync.dma_start(raw["a"][:], src["a"][:]).then_inc(sem["da0"], 16)
            # outputs, earliest-ready first
            wait_ev(nc.sync, "v", "bn_a1")
            nc.sync.dma_start(st_a[:], smv["a"][:]).then_inc(sem["dout_s"], 16)
            wait_ev(nc.sync, "v", "bn_b1")
            nc.sync.dma_start(st_b[:], smv["b"][:]).then_inc(sem["dout_s"], 16)
            wait_ev(nc.sync, "v", "qred1")
            nc.sync.dma_start(qd[:], q_sb[:]).then_inc(sem["dout_s"], 16)
            wait_ev(nc.sync, "v", "cp0_a")
            wait_ev(nc.sync, "a", "cp1_a")
            nc.sync.dma_start(
                ga.rearrange("m p n -> p m n"), g_sb["a"][:]
            ).then_inc(sem["dout_s"], 16)
            nc.sync.wait_ge(sem["dout_s"], 64)

    nc.compile()
    return nc, ev


def _get_program():
    if "nc" not in _CACHE:
        _, ev = _build_program()       # pass 1: record event chain indices
        _CACHE["nc"], _ = _build_program(ev)  # pass 2: real wait values
    return _CACHE["nc"]


LAST_RESULT = None


def kernel(z_a: np.ndarray, z_b: np.ndarray) -> np.ndarray:
    global LAST_RESULT
    import ml_dtypes

    from concourse.bass_utils import run_bass_kernel_spmd

    z_a = np.asarray(z_a, dtype=np.float32)
    z_b = np.asarray(z_b, dtype=np.float32)
    assert z_a.shape == (N, D) and z_b.shape == (N, D)

    nc = _get_program()

    bf = ml_dtypes.bfloat16
    in_maps = []
    for c in range(NCORES):
        sl = slice(c * D_LOCAL, (c + 1) * D_LOCAL)
        in_maps.append(
            {
                "za_t": np.ascontiguousarray(z_a[:, sl].T.astype(bf)),
                "zb_t": np.ascontiguousarray(z_b[:, sl].T.astype(bf)),
            }
        )

    res = run_bass_kernel_spmd(nc, in_maps, core_ids=list(range(NCORES)))
    LAST_RESULT = res

    Ga = np.zeros((2 * P, N), dtype=np.float64)
    Gb = np.zeros((2 * P, N), dtype=np.float64)
    q = np.empty(D, dtype=np.float64)  # c_dd * N
    for c in range(NCORES):
        out = res.results[c]
        Ga += out["ga"].reshape(2 * P, N).astype(np.float64)
        Gb += out["gb"].reshape(2 * P, N).astype(np.float64)
        sta = out["st_a"].astype(np.float64)
        stb = out["st_b"].astype(np.float64)
        mean_a, var_a = sta[:, :, 0], sta[:, :, 1] * (N / (N - 1.0))
        mean_b, var_b = stb[:, :, 0], stb[:, :, 1] * (N / (N - 1.0))
        r = out["qd"].astype(np.float64)  # [P, NT] raw sum_n a*b at (p, i)
        qc = (r - N * mean_a * mean_b) / np.sqrt(var_a * var_b)
        q[c * D_LOCAL : (c + 1) * D_LOCAL] = qc.reshape(D_LOCAL)

    sum_c2 = float((Ga * Gb).sum()) / (N * N)  # sum over ALL (d, e) of c^2
    cdd = q / N
    loss = (
        LAMBDA * (sum_c2 - float((cdd * cdd).sum()))
        + float(((cdd - 1.0) ** 2).sum())
    )
    return np.float32(loss)


if __name__ == "__main__":
    rng = np.random.default_rng(0)
    za = rng.standard_normal((N, D), dtype=np.float32)
    zb = rng.standard_normal((N, D), dtype=np.float32)
    out = kernel(z_a=za, z_b=zb)
    print("kernel output:", out)
